# revision 1
# baseline (speedup 1.0000x reference)
"""DKEPooling Trainium2 kernel (v2).

Per-graph pipeline (d=256, n=512 nodes/graph):
  f = feat + 0.01*noise     (one SWDGE cast DMA + one cast+accum DMA -> bf16;
                             the host pre-scales noise by 0.01)
  C' = f^T f - s^T s / n    (Gram + rank-1 in PSUM; s = colsum f via ones-matmul)
  A  = C'/tr(C')            (trace = diag-mask partials + one fused
                             reduce-and-broadcast matmul with an all-ones block)
  Newton-Schulz via the commuting-polynomial invariant
  T_{k+1} = 0.25 T_k (3I - T_k)^2   (6 matrix products per graph), then an
  8-matvec tail applied to the mean.

Layout: every chain matrix is ONE [128, 512] tile (row-chunks side by side in
the free dim) = exactly one PSUM bank, so each stage is 4 matmuls + 1 evac +
1 DVE combine.  PSUM rule learned on HW: only one OPEN accumulation group per
bank at a time (interleaved groups corrupt each other), so the Gram keeps each
chunk's group open until the rank-1 update closes it.  f32r matmuls also
require dst partition base 0: tail matvec rows land as [W,256] blocks at
partitions 0:W (junk rows are the other graphs' columns), two graphs per bank,
and tiny [4,4] selector matmuls transpose the right row back to column form;
the per-graph output scale cb rides in the v0 selector values.

Sharding: data-parallel over graphs. 8 cores x 16 graphs; no cross-core comm.
"""
import numpy as np

import concourse.bacc as bacc
import concourse.bass as bass
import concourse.mybir as mybir
import concourse.tile as tile
from concourse.bass_utils import run_bass_kernel_spmd

F32 = mybir.dt.float32
BF16 = mybir.dt.bfloat16
F32R = mybir.dt.float32r
ALU = mybir.AluOpType
ACTF = mybir.ActivationFunctionType

N_CORES = 8
D = 256
NPG = 512
B_TOTAL = 128
B_CORE = B_TOTAL // N_CORES      # 16 graphs per core
ROWS_CORE = B_CORE * NPG         # 8192 feat rows per core
W = 4                            # graphs per tail wave
N_WAVES = B_CORE // W

# const tensor layout (f32 [128, 773]):
#   [:, 0:512]   = wide 3I: 3I block at cols 0:128 (chunk0) and 384:512 (chunk1)
#   [:, 512:640] = I128 (diag mask)
#   [:, 640:644] = M32: M32[32b, b] = 1  (row-selector for W=4 graphs)
#   [:, 644:772] = all-ones 128x128 block (fused trace reduce+broadcast)
#   [:, 772:788] = E4SEL: E4[c, 4b+j] = (c == b), c < 4 (tail row selector)
CST_COLS = 788


def _const_arrays():
    import ml_dtypes
    cst = np.zeros((128, CST_COLS), np.float32)
    eye = np.eye(128, dtype=np.float32)
    cst[:, 0:128] = 3.0 * eye
    cst[:, 384:512] = 3.0 * eye
    cst[:, 512:640] = eye
    for b in range(W):
        cst[32 * b, 640 + b] = 1.0
    cst[:, 644:772] = 1.0
    for b in range(W):
        cst[b, 772 + 4 * b: 772 + 4 * (b + 1)] = 1.0
    cstb = np.ones((128, 1), ml_dtypes.bfloat16)
    return cst, cstb


def _r(ap):
    return ap.bitcast(F32R)


def build_module():
    nc = bacc.Bacc(None, target_bir_lowering=False)
    feat_d = nc.declare_dram_parameter("feat", [ROWS_CORE, D], F32, isOutput=False)
    noise_d = nc.declare_dram_parameter("noise", [ROWS_CORE, D], F32, isOutput=False)
    cst_d = nc.declare_dram_parameter("cst", [128, CST_COLS], F32R, isOutput=False)
    cstb_d = nc.declare_dram_parameter("cstb", [128, 1], BF16, isOutput=False)
    out_d = nc.declare_dram_parameter("out", [B_CORE, D], F32, isOutput=True)

    with tile.TileContext(nc) as tc:
        _build_tile(tc, nc, feat_d, noise_d, cst_d, cstb_d, out_d)
    nc.compile()
    return nc


def _build_tile(tc, nc, feat_d, noise_d, cst_d, cstb_d, out_d):
    import contextlib
    import concourse.bass_isa as bass_isa
    ctx = contextlib.ExitStack()
    with ctx:
        g_p = ctx.enter_context(tc.tile_pool(name="gp", bufs=8))
        st0_p = ctx.enter_context(tc.tile_pool(name="st0p", bufs=2))
        gc_p = ctx.enter_context(tc.tile_pool(name="gcp", bufs=4))
        mats_p = ctx.enter_context(tc.tile_pool(name="mats", bufs=9))
        chain_p = ctx.enter_context(tc.tile_pool(name="chain", bufs=4))
        small_p = ctx.enter_context(tc.tile_pool(name="small", bufs=12))
        tail_p = ctx.enter_context(tc.tile_pool(name="tailp", bufs=3))
        wave_p = ctx.enter_context(tc.tile_pool(name="wavep", bufs=2))
        cst_p = ctx.enter_context(tc.tile_pool(name="cstp", bufs=1))
        psGram = ctx.enter_context(tc.tile_pool(name="psGram", bufs=2, space="PSUM"))
        psStage = ctx.enter_context(tc.tile_pool(name="psStage", bufs=2, space="PSUM"))
        psS = ctx.enter_context(tc.tile_pool(name="psS", bufs=1, space="PSUM"))
        psRows = ctx.enter_context(tc.tile_pool(name="psRows", bufs=1, space="PSUM"))
        psTpc = ctx.enter_context(tc.tile_pool(name="psTpc", bufs=1, space="PSUM"))

        onesb = cst_p.tile([128, 1], BF16, tag="onesb", name="onesb_sb")
        nc.gpsimd.dma_start(onesb, cstb_d[:, :])
        cst = cst_p.tile([128, CST_COLS], F32R, tag="cst", name="cst_sb")
        cst_loaded = [False]

        def load_cst():
            if not cst_loaded[0]:
                cst_loaded[0] = True
                nc.scalar.dma_start(cst, cst_d[:, :])

        cst3I = cst.bitcast(F32)[:, 0:512]
        I128 = cst.bitcast(F32)[:, 512:640]
        M32 = cst[:, 640:640 + W]          # f32r: matmul selector
        M32f = cst.bitcast(F32)[:, 640:640 + W]
        ones128 = cst.bitcast(F32)[:, 644:772]
        E4SEL = cst[:, 772:788]

        n_zeroed = {"s": 0}
        hook = globals().get("_DEBUG_HOOK", None) or (lambda name, ap: None)

        def mm256(dst_ps, L, R):
            """dst = L @ R for [128,512]-layout symmetric matrices (f32r APs)."""
            for m in range(2):
                for k in range(2):
                    nc.tensor.matmul(
                        dst_ps[:, m * D:(m + 1) * D],
                        L[:, k * D + m * 128: k * D + (m + 1) * 128],
                        R[:, k * D:(k + 1) * D],
                        start=(k == 0), stop=(k == 1))

        def phase_a1(g, s_ps, b):
            """Load graph g; column sums s into row 32b of the wave s bank."""
            gb = g_p.tile([128, 4 * D], BF16, tag="g", name=f"g_{g}")
            src = feat_d[g * NPG:(g + 1) * NPG, :].rearrange("(c p) d -> p c d", p=128)
            nsrc = noise_d[g * NPG:(g + 1) * NPG, :].rearrange("(c p) d -> p c d", p=128)
            if g < 2:
                # wave 0: HWDGE per-chunk f32 loads (SP + ACT issue in
                # parallel, no Pool descriptor serialization) + DVE combine;
                # SWDGE cast+accum loads would gate the first Gram by ~7us.
                ft = st0_p.tile([128, 4 * D], F32, tag="ft0", name=f"ft0_{g}")
                nz = st0_p.tile([128, 4 * D], F32, tag="nz0", name=f"nz0_{g}")
                for c in range(4):
                    nc.sync.dma_start(ft[:, c * D:(c + 1) * D], src[:, c, :])
                    nc.scalar.dma_start(nz[:, c * D:(c + 1) * D], nsrc[:, c, :])
                for c in range(4):
                    nc.vector.tensor_add(gb[:, c * D:(c + 1) * D],
                                         ft[:, c * D:(c + 1) * D],
                                         nz[:, c * D:(c + 1) * D])
            else:
                nc.gpsimd.dma_start(gb, src)
                nc.gpsimd.dma_start(gb, nsrc, accum_op=ALU.add)
            for k in range(4):
                nc.tensor.matmul(s_ps[32 * b:32 * b + 1, 0:256], onesb,
                                 gb[:, k * D:(k + 1) * D],
                                 start=(k == 0), stop=(k == 3),
                                 tile_position=(0, 32 * b))
            return gb

        def phase_a2(g, gb, SB4, SBn4, b):
            """Gram + rank-1 into one [128,512] PSUM bank; per chunk m the
            accumulation group stays open from the first k-matmul until the
            rank-1 closes it (one open group per bank at a time)."""
            G = psGram.tile([128, 512], F32, tag="G", name=f"G_{g}")
            for m in range(2):
                for k in range(4):
                    nc.tensor.matmul(
                        G[:, m * D:(m + 1) * D],
                        gb[:, k * D + m * 128: k * D + (m + 1) * 128],
                        gb[:, k * D:(k + 1) * D],
                        start=(k == 0), stop=False)
                nc.tensor.matmul(G[:, m * D:(m + 1) * D],
                                 SBn4[32 * b:32 * b + 1, m * 128:(m + 1) * 128],
                                 SB4[32 * b:32 * b + 1, :],
                                 start=False, stop=True,
                                 tile_position=(32 * b, 0))
            return {"gb": gb, "G": G}

        def phase_b(wave, sts, s_ps, S4, SB4, SBn4):
            """rank-1 correction + trace + NS chain, stage-major across the
            wave's W graphs so independent graphs interleave on every engine."""
            gs = [wave * W + b for b in range(W)]
            Gcs, As, mats = [], [], [{} for _ in range(W)]

            # C' evacuation
            for b in range(W):
                G = sts[b]["G"]
                Gc = gc_p.tile([128, 512], F32R, tag="Gc", name=f"Gc_{gs[b]}")
                nc.scalar.copy(Gc, G)
                if gs[b] == 0:
                    hook("gc", Gc.bitcast(F32)[:, :])
                Gcs.append(Gc)

                        # trace: diag-mask partial sums (DVE) -> reduce + broadcast on PE
            # (via the widened s-bank: tr at [0, 256+b], bc at [:, 260+b])
            rcpbs, rcp2bs = [], []
            for b in range(W):
                g = gs[b]
                scr = small_p.tile([128, 128], F32, tag="scr", name=f"scr_{g}")
                dg = small_p.tile([128, 2], F32, tag="dg", name=f"dg_{g}")
                for m in range(2):
                    nc.vector.scalar_tensor_tensor(
                        scr, Gcs[b].bitcast(F32)[:, m * D + m * 128: m * D + (m + 1) * 128],
                        1.0, I128, ALU.mult, ALU.mult, accum_out=dg[:, m:m + 1])
                dgs = small_p.tile([128, 1], F32, tag="dgs", name=f"dgs_{g}")
                nc.vector.tensor_add(dgs, dg[:, 0:1], dg[:, 1:2])
                nc.tensor.matmul(s_ps[:, 260 + 4 * b:261 + 4 * b], ones128, dgs,
                                 start=True, stop=True)
                bc = s_ps[:, 260 + 4 * b:261 + 4 * b]
                rcpb = small_p.tile([128, 1], F32, tag="rcpb", name=f"rcpb_{g}")
                nc.vector.reciprocal(rcpb, bc)
                rcp2b = small_p.tile([128, 1], F32, tag="rcp2b", name=f"rcp2b_{g}")
                nc.vector.tensor_mul(rcp2b, rcpb, rcpb)
                # cb = sqrt(trc/(n-1)) * 0.03125/n, broadcast over partitions
                sq = small_p.tile([128, 1], F32, tag="sq", name=f"sq_{g}")
                nc.scalar.activation(sq, bc, ACTF.Sqrt, scale=1.0 / (NPG - 1))
                cb = small_p.tile([128, 1], F32, tag="cb", name=f"cb_{g}")
                nc.vector.tensor_scalar_mul(cb, sq, 0.03125 / NPG)
                rcpbs.append(rcpb)
                rcp2bs.append(rcp2b)
                mats[b]["cb"] = cb

            for b in range(W):
                A = mats_p.tile([128, 512], F32R, tag="A", name=f"A_{gs[b]}")
                nc.vector.tensor_scalar_mul(A, Gcs[b].bitcast(F32), rcpbs[b])
                if gs[b] == 0:
                    hook("a", A.bitcast(F32)[:, :])
                As.append(A)
                mats[b]["A"] = A

            # A2 (normalized via rcp^2 at evac)
            sta = [psStage.tile([128, 512], F32, tag="st", name=f"a2_{gs[b]}")
                   for b in range(W)]
            for b in range(W):
                mm256(sta[b], Gcs[b], Gcs[b])
            A2ns, W1s, V0s = [], [], []
            for b in range(W):
                A2n = chain_p.tile([128, 512], F32, tag="A2n", name=f"A2n_{gs[b]}")
                nc.scalar.activation(A2n, sta[b], ACTF.Copy, scale=rcp2bs[b])
                A2ns.append(A2n)
            for b in range(W):
                W1 = chain_p.tile([128, 512], F32R, tag="W1", name=f"W1_{gs[b]}")
                nc.vector.scalar_tensor_tensor(W1, As[b], 3.0, A2ns[b],
                                               ALU.mult, ALU.subtract)
                W1s.append(W1)
                mats[b]["W1"] = W1
                V0 = chain_p.tile([128, 512], F32R, tag="V0", name=f"V0_{gs[b]}")
                nc.vector.scalar_tensor_tensor(V0, As[b], -1.0, cst3I,
                                               ALU.mult, ALU.add)
                V0s.append(V0)

            stp = [psStage.tile([128, 512], F32, tag="st", name=f"p_{gs[b]}")
                   for b in range(W)]
            T1s, V1s = [], []
            for b in range(W):
                mm256(stp[b], W1s[b], V0s[b])
            for b in range(W):
                T1 = mats_p.tile([128, 512], F32R, tag="T1", name=f"T1_{gs[b]}")
                nc.scalar.mul(T1, stp[b], 0.25)
                T1s.append(T1)
                mats[b]["T1"] = T1
            for b in range(W):
                V1 = chain_p.tile([128, 512], F32R, tag="V1", name=f"V1_{gs[b]}")
                nc.vector.scalar_tensor_tensor(V1, T1s[b], -1.0, cst3I,
                                               ALU.mult, ALU.add)
                V1s.append(V1)
                mats[b]["V1"] = V1

            stq = [psStage.tile([128, 512], F32, tag="st", name=f"q_{gs[b]}")
                   for b in range(W)]
            Qbs = []
            for b in range(W):
                mm256(stq[b], T1s[b], V1s[b])
            for b in range(W):
                Qb = chain_p.tile([128, 512], F32R, tag="Qb", name=f"Qb_{gs[b]}")
                nc.scalar.copy(Qb, stq[b])
                Qbs.append(Qb)

            str_ = [psStage.tile([128, 512], F32, tag="st", name=f"rr_{gs[b]}")
                    for b in range(W)]
            T2s, V2s = [], []
            for b in range(W):
                mm256(str_[b], Qbs[b], V1s[b])
            for b in range(W):
                T2 = mats_p.tile([128, 512], F32R, tag="T2", name=f"T2_{gs[b]}")
                nc.scalar.mul(T2, str_[b], 0.25)
                T2s.append(T2)
                mats[b]["T2"] = T2
            for b in range(W):
                V2 = chain_p.tile([128, 512], F32R, tag="V2", name=f"V2_{gs[b]}")
                nc.vector.scalar_tensor_tensor(V2, T2s[b], -1.0, cst3I,
                                               ALU.mult, ALU.add)
                V2s.append(V2)
                mats[b]["V2"] = V2

            sts5 = [psStage.tile([128, 512], F32, tag="st", name=f"s5_{gs[b]}")
                    for b in range(W)]
            Sbs = []
            for b in range(W):
                mm256(sts5[b], T2s[b], V2s[b])
            for b in range(W):
                Sb = chain_p.tile([128, 512], F32R, tag="Sb", name=f"Sb_{gs[b]}")
                nc.vector.tensor_copy(Sb, sts5[b])
                Sbs.append(Sb)

            stu = [psStage.tile([128, 512], F32, tag="st", name=f"u_{gs[b]}")
                   for b in range(W)]
            for b in range(W):
                mm256(stu[b], Sbs[b], V2s[b])
            for b in range(W):
                T3 = mats_p.tile([128, 512], F32R, tag="T3", name=f"T3_{gs[b]}")
                nc.scalar.mul(T3, stu[b], 0.25)
                if gs[b] == 0:
                    hook("t3", T3.bitcast(F32)[:, :])
                mats[b]["T3"] = T3
            return mats

        def tail_step(si, wave, cur, mats, kind, v0c3, tailidx, kind_x=None):
            """One tail step for all W graphs.

            rows land at 32-aligned PSUM partitions, transposed back to
            column form [128, 2W] with the M32 selector."""
            xkey = kind_x if kind_x is not None else {
                0: "T3", 1: "T3", 2: "T3", 3: "T3",
                4: "T2", 5: "T1", 6: "A", 7: "A"}[si]
            # f32r matmuls require dst partition base 0: each graph's row
            # lands as a [W,256] block at partitions 0:W (junk rows = other
            # cur columns vs X_b), two graphs per PSUM bank.
            rows = [psRows.tile([W, 512], F32, tag=f"rows{h}",
                                name=f"rows{h}_{wave}_{si}") for h in range(2)]
            for b in range(W):
                X = mats[b][xkey]
                dst = rows[b // 2][:, (b % 2) * D:(b % 2 + 1) * D]
                for k in range(2):
                    nc.tensor.matmul(dst, cur[:, k * W:(k + 1) * W],
                                     X[:, k * D:(k + 1) * D],
                                     start=(k == 0), stop=(k == 1))
            if kind == "final":
                for h in range(2):
                    osb = tail_p.tile([W, 512], F32, tag=f"osb{h}",
                                      name=f"osb{h}_{wave}")
                    nc.scalar.copy(osb, rows[h])
                    for j in range(2):
                        b = 2 * h + j
                        nc.sync.dma_start(out_d[wave * W + b:wave * W + b + 1, :],
                                          osb[b:b + 1, j * D:(j + 1) * D])
                return None
            usb = []
            for h in range(2):
                u = tail_p.tile([W, 512], F32R, tag=f"usb{h}",
                                name=f"usb{h}_{wave}_{si}")
                if h == 0:
                    nc.scalar.copy(u, rows[h])
                else:
                    nc.vector.tensor_copy(u, rows[h])
                usb.append(u)
            tpc = psTpc.tile([128, 40], F32, tag="tpc", name=f"tpc_{wave}_{si}")
            for b in range(W):
                for m in range(2):
                    nc.tensor.matmul(tpc[:, (m * W + b) * 4:(m * W + b + 1) * 4],
                                     usb[b // 2][0:W, (b % 2) * D + m * 128:
                                                 (b % 2) * D + (m + 1) * 128],
                                     E4SEL[0:W, 4 * b:4 * (b + 1)],
                                     start=True, stop=True)
            ucs = tpc[:, 0:32].rearrange("p (c j) -> p c j", j=4)[:, :, 0]
            nxt = tail_p.tile([128, 2 * W], F32R, tag="cur",
                              name=f"cur_{wave}_{si}")
            if kind == "comb":
                nc.vector.scalar_tensor_tensor(nxt, cur, 3.0, ucs,
                                               ALU.mult, ALU.subtract)
            elif kind == "a3":
                nc.vector.scalar_tensor_tensor(nxt, ucs, -0.25, v0c3,
                                               ALU.mult, ALU.add)
            return nxt

        def build_F(wave, mats):
            """Collapse the 8-step tail into one matrix per graph:
            out = W1 (V1 V2) (V3 V4) v0, all factors commute (polys of A).
            Reuses dead tile tags so SBUF does not grow."""
            gs = [wave * W + b for b in range(W)]
            V3s, C2bs, Mbs, U4es, V4s, C34bs, F1bs, Fbs = ([] for _ in range(8))
            for b in range(W):
                V3 = chain_p.tile([128, 512], F32R, tag="V0", name=f"V3_{gs[b]}")
                nc.vector.scalar_tensor_tensor(V3, mats[b]["T3"], -1.0, cst3I,
                                               ALU.mult, ALU.add)
                V3s.append(V3)
            stc2 = [psStage.tile([128, 512], F32, tag="st", name=f"c2_{gs[b]}")
                    for b in range(W)]
            for b in range(W):
                mm256(stc2[b], mats[b]["V1"], mats[b]["V2"])
            for b in range(W):
                C2b = chain_p.tile([128, 512], F32R, tag="V1", name=f"C2b_{gs[b]}")
                nc.scalar.copy(C2b, stc2[b])
                C2bs.append(C2b)
            stm = [psStage.tile([128, 512], F32, tag="st", name=f"m34_{gs[b]}")
                   for b in range(W)]
            for b in range(W):
                mm256(stm[b], mats[b]["T3"], V3s[b])
            for b in range(W):
                Mb = chain_p.tile([128, 512], F32R, tag="Qb", name=f"Mb_{gs[b]}")
                nc.scalar.copy(Mb, stm[b])
                Mbs.append(Mb)
            stu4 = [psStage.tile([128, 512], F32, tag="st", name=f"u4_{gs[b]}")
                    for b in range(W)]
            for b in range(W):
                mm256(stu4[b], Mbs[b], V3s[b])
            for b in range(W):
                U4e = chain_p.tile([128, 512], F32, tag="A2n", name=f"U4e_{gs[b]}")
                nc.scalar.mul(U4e, stu4[b], -0.25)
                U4es.append(U4e)
            for b in range(W):
                V4 = chain_p.tile([128, 512], F32R, tag="Sb", name=f"V4_{gs[b]}")
                nc.vector.scalar_tensor_tensor(V4, U4es[b], 1.0, cst3I,
                                               ALU.mult, ALU.add)
                V4s.append(V4)
            stc34 = [psStage.tile([128, 512], F32, tag="st", name=f"c34_{gs[b]}")
                     for b in range(W)]
            for b in range(W):
                mm256(stc34[b], V3s[b], V4s[b])
            for b in range(W):
                C34b = mats_p.tile([128, 512], F32R, tag="T1", name=f"C34b_{gs[b]}")
                nc.scalar.copy(C34b, stc34[b])
                C34bs.append(C34b)
            stf1 = [psStage.tile([128, 512], F32, tag="st", name=f"f1_{gs[b]}")
                    for b in range(W)]
            for b in range(W):
                mm256(stf1[b], C2bs[b], C34bs[b])
            for b in range(W):
                F1b = chain_p.tile([128, 512], F32R, tag="V2", name=f"F1b_{gs[b]}")
                nc.scalar.copy(F1b, stf1[b])
                F1bs.append(F1b)
            stf = [psStage.tile([128, 512], F32, tag="st", name=f"f_{gs[b]}")
                   for b in range(W)]
            for b in range(W):
                mm256(stf[b], mats[b]["W1"], F1bs[b])
            for b in range(W):
                Fb = mats_p.tile([128, 512], F32R, tag="A", name=f"Fb_{gs[b]}")
                nc.scalar.copy(Fb, stf[b])
                mats[b]["F"] = Fb
                Fbs.append(Fb)

        pending_tail = []

        def run_pending_tail():
            if not pending_tail:
                return
            twave, tmats, tv0c, tv0c3 = pending_tail.pop(0)
            cur = tv0c
            kinds = ["comb", "comb", "a3", "comb", "comb", "comb", "comb",
                     "final"]
            for si in range(8):
                cur = tail_step(si, twave, cur, tmats, kinds[si], tv0c3, si)

        for wave in range(N_WAVES):
            s_ps = psS.tile([128, 276], F32, tag="s", name=f"s_{wave}")
            if n_zeroed["s"] < 1:
                n_zeroed["s"] += 1
                nc.scalar.memzero(s_ps)
            gbs = []
            for b in range(W):
                g = wave * W + b
                gbs.append(phase_a1(g, s_ps, b))
            load_cst()
            run_pending_tail()
            # s evac + bf16 row tiles for the rank-1 update
            S4 = wave_p.tile([128, 256], F32R, tag="S4", name=f"S4_{wave}")
            nc.scalar.copy(S4, s_ps[:, 0:256])
            SB4 = wave_p.tile([128, 256], BF16, tag="SB4", name=f"SB4_{wave}")
            nc.scalar.copy(SB4, s_ps[:, 0:256])
            SBn4 = wave_p.tile([128, 256], BF16, tag="SBn4", name=f"SBn4_{wave}")
            nc.vector.tensor_scalar_mul(SBn4, SB4, -1.0 / NPG)
            sts = []
            for b in range(W):
                g = wave * W + b
                sts.append(phase_a2(g, gbs[b], SB4, SBn4, b))
            mats = phase_b(wave, sts, s_ps, S4, SB4, SBn4)

            # v0 columns via cb-valued selector
            E = wave_p.tile([128, W], F32R, tag="E", name=f"E_{wave}")
            for b in range(W):
                nc.vector.scalar_tensor_tensor(E[:, b:b + 1], mats[b]["cb"], 1.0,
                                               M32f[:, b:b + 1], ALU.mult, ALU.mult)
            tpv = psTpc.tile([128, 40], F32, tag="tpc", name=f"tpv_{wave}")
            for m in range(2):
                nc.tensor.matmul(tpv[:, 32 + m * W:32 + (m + 1) * W],
                                 S4[:, m * 128:(m + 1) * 128],
                                 E, start=True, stop=True)
            v0c = tail_p.tile([128, 2 * W], F32R, tag="cur", name=f"v0c_{wave}")
            nc.scalar.copy(v0c, tpv[:, 32:40])
            v0c3 = tail_p.tile([128, 2 * W], F32R, tag="v0c3", name=f"v0c3_{wave}")
            nc.vector.tensor_scalar_mul(v0c3, v0c, 3.0)

            if wave == 0:
                hook("s", s_ps[:, :])
                hook("v0", v0c.bitcast(F32)[:, :])
            if wave == N_WAVES - 1:
                build_F(wave, mats)
                tail_step(7, wave, v0c, mats, "final", v0c3, 0, kind_x="F")
            else:
                pending_tail.append((wave, mats, v0c, v0c3))
        run_pending_tail()


_CACHED_NC = None


def _get_nc():
    global _CACHED_NC
    if _CACHED_NC is None:
        _CACHED_NC = build_module()
    return _CACHED_NC


def _run(feat, noise, **spmd_kwargs):
    feat = np.ascontiguousarray(np.asarray(feat), dtype=np.float32)
    noise01 = np.asarray(noise, dtype=np.float32) * np.float32(0.01)
    noise01 = np.ascontiguousarray(noise01)
    cst, cstb = _const_arrays()
    nc = _get_nc()
    in_maps = []
    for c in range(N_CORES):
        in_maps.append({
            "feat": feat[c * ROWS_CORE:(c + 1) * ROWS_CORE],
            "noise": noise01[c * ROWS_CORE:(c + 1) * ROWS_CORE],
            "cst": cst,
            "cstb": cstb,
        })
    return run_bass_kernel_spmd(nc, in_maps, list(range(N_CORES)), **spmd_kwargs)


def kernel(feat, noise, n_per_graph):
    assert int(n_per_graph) == NPG
    try:
        res = _run(feat, noise)
    except Exception:
        # the axon device occasionally reports a transient unrecoverable
        # state; one retry usually succeeds
        res = _run(feat, noise)
    return np.concatenate([res.results[c]["out"] for c in range(N_CORES)], axis=0)

